# revision 34
# baseline (speedup 1.0000x reference)
"""Neural CDE forward pass on 8 Trainium2 NeuronCores (Bass/Tile).

Math (per batch element b):
    z0 = u0 @ Wi + bi                                   [64]
    for t in 0..164:
        h  = relu(z @ W1 + b1)                          [128]
        f  = tanh(h @ W2 + b2)                          [512] -> [64, 8]
        z += einsum('hi,i->h', f, dx_t)                 dx_t = coeffs[t+1]-coeffs[t]
    out_t = z_t @ Wr + br  for every t (166 values)

Numerics (hardware-measured, see git history of this docstring):
  - The scan is chaotic: errors amplify ~1.05x/step (~3000x over 165 steps).
  - fp32 matmul: exact-grade but 4 cycles/row; float32r: operands rounded
    to ~12 mantissa bits (1.4e-4 rel/step) at 1 cycle/row.
  - Hybrid phase split: steps t < T0=59 run fp32 mm2/reduce, t >= T0 run
    f32r.  Final error ~1e-2 rel (gate 2e-2).  T0=59 is the measured knee;
    mm1 (the z state stream) stays fp32 in both phases.

Kernel design (per core, batch shard B=512 in NCHAIN=2 chains of Bc=256):
  - State z [66, Bc] fp32: rows 0..63 state, row 64 == 1.0 (carries b1
    into mm1: stationary w1b[65,128] = [W1; b1], moving z[0:65]), row 65 =
    running readout out_t = z_t @ Wr + br.  The 1.0 row means relu needs
    NO fused bias, so it can run on any engine.  The reduce matrices S_j
    [128, 66] get a zero column at 64 (keeps the 1.0 row fixed) and the
    Wr column at 65 (readout rides the reduce for free).
  - States are slotted: state s lives at z_st[c][:, s % NSLOT, :].  Output
    row 65 drains with ONE DMA per OUTB=4 states per chain, and dx loads
    arrive DXB=4 steps per DMA (HWDGE fixed cost ~625ns/DMA; batching
    keeps the SP queue + HWDGE device off the critical path).
  - BOTH phases emit per-chain ops (knobs esplit/lsplit): each chain is
    an independent recurrence pipeline (mm1/relu/mm2/tanh/gmul/reduce/add
    per chain at [*, 256]); the two pipelines interleave on the engines,
    hiding each other's serial z->h->f->g->e->z latency.  HW-measured
    11.77 -> 10.84 us/step (fp32 phase) and 8.3 -> 6.98 us/step (f32r
    phase) vs the fused forms.  PSUM tiles keep the fused [.., NCHAIN,
    Bc] shape with per-chain half-views so phases share one PSUM budget
    (14KB of 16KB).
  - Engine knobs spread elementwise work: relu/gmul/z-add per chain on
    vector vs gpsimd vs scalar (tuned via the cost-model TimelineSim,
    verified on hardware).  GPSIMD has no PSUM port, so PSUM readers
    (relu, add) stay on vector/scalar.
  - Per-matmul HW overhead is ~130-160ns (serialized weight load + SEQ
    dispatch, NOT modeled by TimelineSim), which is why designs that add
    matmuls (redsplit, esplit+redsplit at 26 MM/step) lost on hardware
    despite winning in the cost model; see the redsplit knob (default
    off) and the session memory for the full failure record.
"""

import numpy as np

IN_CH = 8
HID = 64
MLP_W = 128
OUT = 1
B_FULL, T = 4096, 166
NSTEP = T - 1
N_CORES = 8
B = B_FULL // N_CORES  # 512
NBANK = 4
HID1 = HID + 2  # 64 state rows + const-one row (64) + readout row (65)
ROW_ONE = HID
ROW_OUT = HID + 1

T0 = 59  # steps < T0 run fp32 matmuls; steps >= T0 run f32r
NCHAIN = 2
Bc = B // NCHAIN  # 256
DXB = 4  # scan steps per dx DMA block (batched to amortize HWDGE overhead)
NDXBLK = (NSTEP + DXB - 1) // DXB  # 42 (last block zero-padded)
DXBLK_PREFETCH = 2
NSLOT = 8  # z state slots per chain (state s lives at slot s % NSLOT)
OUTB = 4  # states per output DMA (4 divides NSLOT so slot runs stay contiguous)
REPEAT = 1

# engine assignment knobs ("vector" | "gpsimd" | "scalar" where noted)
KNOBS = dict(
    t0=T0,
    lsplit=True,                      # per-chain emission in the f32r phase
    relu_e="scalar",                  # fp32-phase relu engine
    # PSUM readers (relu, z-add) are restricted to vector/scalar — GPSIMD
    # has no PSUM port.  gmul reads SBUF only, so it may use gpsimd.
    # skew mode: c0's relu is on the spine (keep off ACT's tanh queue);
    # the last banks' wide gmuls gate the reduce tail (keep off GPSIMD)
    relu_l=("scalar", "vector"),      # relu engine per chain
    gmul_e=("vector", "gpsimd", "vector", "gpsimd"),  # wide gmuls, per bank
    gmul_l=(("vector", "gpsimd", "vector", "gpsimd"),
            ("gpsimd", "vector", "gpsimd", "vector")),  # f32r, [chain][bank]
    add_e=("vector", "vector"),       # per chain, both phases
    # fp32-phase reduce split: e = S.T g_hi + S.T g_lo with g_hi=round11(g)
    # (f32r store rounding), g_lo = g - g_hi.  Replaces 4 fp32 reduce
    # matmuls (4 cyc/row) with 8 f32r ones (1 cyc/row); error ~2^-24.
    redsplit=False,
    esplit=True,                      # per-chain emission in the fp32 phase
    mm2fuse=False,                     # fuse mm2 across chains in split steps
    # skewed fusion: chain 1 runs one step behind chain 0 so every wide op
    # processes [c0@k | c1@k-1] — fused op counts (10 MMs/step) with no
    # cross-chain lockstep (only c0's mm1/relu/add are on the spine; c1's
    # per-chain ops have a full step of slack).  Value-identical per chain.
    skew=False,
    ghi_e=("scalar", "vector", "scalar", "vector"),   # g_hi copy engine/bank
    glo_e=("vector", "gpsimd", "vector", "gpsimd"),   # g_lo sub engine/bank
)

_CACHE = {}


def _build_bass(repeat=1, knobs=None):
    from contextlib import ExitStack

    import concourse.tile as tile
    from concourse import bacc, mybir

    kn = dict(KNOBS)
    if knobs:
        kn.update(knobs)

    f32 = mybir.dt.float32
    f32r = mybir.dt.float32r
    f16 = mybir.dt.float16
    AF = mybir.ActivationFunctionType

    nc = bacc.Bacc("TRN2", target_bir_lowering=False, debug=False)

    u0t = nc.dram_tensor("u0t", [IN_CH, B], f32, kind="ExternalInput")
    dxt = nc.dram_tensor("dxt", [NDXBLK, IN_CH, DXB, B], f32,
                         kind="ExternalInput")
    w1b = nc.dram_tensor("w1b", [HID + 1, MLP_W], f32, kind="ExternalInput")
    w2 = nc.dram_tensor("w2", [MLP_W, NBANK, 128], f32, kind="ExternalInput")
    b2 = nc.dram_tensor("b2", [128, NBANK], f32, kind="ExternalInput")
    wi = nc.dram_tensor("wi", [IN_CH, HID1], f32, kind="ExternalInput")
    smat = nc.dram_tensor("smat", [128, NBANK, HID1], f32,
                          kind="ExternalInput")
    outp = nc.dram_tensor("outp", [T, B], f32, kind="ExternalOutput")

    def eng(name):
        return getattr(nc, name)

    def relu_op(engine, dst, src):
        if engine == "scalar":
            nc.scalar.activation(dst, src, AF.Relu)
        else:
            eng(engine).tensor_relu(dst, src)

    with tile.TileContext(nc) as tc, ExitStack() as ctx:
        const = ctx.enter_context(tc.tile_pool(name="const", bufs=1))
        hpool = ctx.enter_context(tc.tile_pool(name="hpool", bufs=kn.get("hbufs", 2)))
        fpool = ctx.enter_context(tc.tile_pool(name="fpool", bufs=kn.get("fbufs", 2)))
        gpool = ctx.enter_context(tc.tile_pool(name="gpool", bufs=kn.get("gbufs", 3)))
        dxpool = ctx.enter_context(tc.tile_pool(name="dxpool", bufs=3))
        psum_h = ctx.enter_context(tc.tile_pool(name="psum_h", bufs=kn.get("phbufs", 2), space="PSUM"))
        psum_f = ctx.enter_context(tc.tile_pool(name="psum_f", bufs=2, space="PSUM"))
        psum_e = ctx.enter_context(tc.tile_pool(name="psum_e", bufs=2, space="PSUM"))

        w1b_sb = const.tile([HID + 1, MLP_W], f32)
        nc.sync.dma_start(w1b_sb[:], w1b[:])
        w2_sb = const.tile([MLP_W, NBANK, 128], f32)
        nc.sync.dma_start(w2_sb[:], w2[:])
        b2_sb = const.tile([128, NBANK], f32)
        nc.sync.dma_start(b2_sb[:], b2[:])
        wi_sb = const.tile([IN_CH, HID1], f32)
        nc.sync.dma_start(wi_sb[:], wi[:])
        s_sb = const.tile([128, NBANK, HID1], f32)
        nc.sync.dma_start(s_sb[:], smat[:])
        s_sb_r = const.tile([128, NBANK, HID1], f32r, name="s_sb_r")
        nc.vector.tensor_copy(s_sb_r[:], s_sb[:])
        u0t_sb = const.tile([IN_CH, B], f32)
        nc.sync.dma_start(u0t_sb[:], u0t[:])

        # f32r alias of W2 for the late phase (PE rounds internally; the
        # DVE copy applies the same rounding, value-identical)
        w2_13 = const.tile([MLP_W, NBANK, 128], f32r, name="w2_13")
        nc.vector.tensor_copy(w2_13[:], w2_sb[:])

        # Slotted state: state s lives at z_st[c][:, s % NSLOT, :]
        z_st = [
            const.tile([HID1, NSLOT, Bc], f32, name=f"z_st{c}")
            for c in range(NCHAIN)
        ]
        dx_blks = {}
        g_banks = [None] * NBANK

        def init_chains():
            z0_ps = psum_e.tile([HID1, NCHAIN, Bc], f32, tag="e_ps",
                                name="z0_ps")
            for c in range(NCHAIN):
                cs = slice(c * Bc, (c + 1) * Bc)
                nc.tensor.matmul(
                    z0_ps[:, c, :], wi_sb[:], u0t_sb[:, cs],
                    start=True, stop=True
                )
                nc.vector.tensor_copy(z_st[c][:, 0, :], z0_ps[:, c, :])

        def drain_chain(c, s_hi):
            """one DMA per OUTB states once state s_hi closes a group (or
            is the last state)"""
            if s_hi % OUTB == OUTB - 1 or s_hi == NSTEP:
                s_lo = (s_hi // OUTB) * OUTB
                sl = s_lo % NSLOT
                n = s_hi - s_lo + 1
                cs = slice(c * Bc, (c + 1) * Bc)
                nc.sync.dma_start(
                    outp[s_lo : s_hi + 1, cs],
                    z_st[c][ROW_OUT : ROW_OUT + 1, sl : sl + n, :],
                )

        def drain_out(t):
            for c in range(NCHAIN):
                drain_chain(c, t + 1)

        def step_fused(t, lo):
            """both chains fused into wide ops (fp32 phase: PE-bound)"""
            h_tile = hpool.tile([MLP_W, NCHAIN, Bc], f32r if lo else f32,
                                tag="h_r" if lo else "h_f", name="h_tile")
            for c in range(NCHAIN):
                h_ps = psum_h.tile([MLP_W, Bc], f32, tag="h_ps", name="h_ps")
                nc.tensor.matmul(
                    h_ps[:], w1b_sb[:], z_st[c][0 : HID + 1, t % NSLOT, :],
                    start=True, stop=True
                )
                relu_op(kn["relu_e"], h_tile[:, c, :], h_ps[:])
            dx_sb = dx_blks[t // DXB][:, t % DXB, :]
            w2_use = w2_13 if lo else w2_sb
            for j in range(NBANK):
                f_ps = psum_f.tile([128, NCHAIN, Bc], f32, tag=f"f_ps{j}",
                                   bufs=1, name=f"f_ps{j}")
                nc.tensor.matmul(f_ps[:], w2_use[:, j, :], h_tile[:],
                                 start=True, stop=True)
                f_sb = fpool.tile([128, NCHAIN, Bc], f16 if lo else f32,
                                  tag=f"f_r{j}" if lo else f"f_f{j}",
                                  name=f"f_sb{j}")
                nc.scalar.activation(
                    f_sb[:], f_ps[:], AF.Tanh, bias=b2_sb[:, j : j + 1]
                )
                g_sb = gpool.tile([128, NCHAIN, Bc], f32r if lo else f32,
                                  tag=f"g_r{j}" if lo else f"g_f{j}",
                                  name=f"g_sb{j}")
                eng(kn["gmul_e"][j]).tensor_mul(g_sb[:], f_sb[:], dx_sb[:])
                g_banks[j] = g_sb
            e_ps = psum_e.tile([HID1, NCHAIN, Bc], f32, tag="e_ps",
                               name="e_ps")
            if not lo and kn["redsplit"]:
                # hi/lo split: both reduce operands f32r (1 cyc/row) with
                # fp32-grade accuracy.  g_hi = round11(g) via f32r store
                # rounding (all engines round identically, probe-verified);
                # g_lo = g - g_hi is exact (Sterbenz) and rounds at store
                # to ~2^-24 of g.  Emission order: all g_hi ops first so
                # engine FIFOs serve the spine before the trailing lo path.
                g_his, g_los = [], []
                for j in range(NBANK):
                    g_hi = gpool.tile([128, NCHAIN, Bc], f32r,
                                      tag=f"ghi{j}", name=f"g_hi{j}")
                    ge = kn["ghi_e"][j]
                    if ge == "scalar":
                        nc.scalar.copy(g_hi[:], g_banks[j][:])
                    else:
                        eng(ge).tensor_copy(g_hi[:], g_banks[j][:])
                    g_his.append(g_hi)
                    nc.tensor.matmul(e_ps[:], s_sb_r[:, j, :], g_hi[:],
                                     start=j == 0, stop=False)
                for j in range(NBANK):
                    g_lo = gpool.tile([128, NCHAIN, Bc], f32r,
                                      tag=f"glo{j}", name=f"g_lo{j}")
                    eng(kn["glo_e"][j]).tensor_sub(
                        g_lo[:], g_banks[j][:], g_his[j][:]
                    )
                    g_los.append(g_lo)
                for j in range(NBANK):
                    nc.tensor.matmul(e_ps[:], s_sb_r[:, j, :], g_los[j][:],
                                     start=False, stop=j == NBANK - 1)
            else:
                s_use = s_sb_r if lo else s_sb
                for j in range(NBANK):
                    nc.tensor.matmul(e_ps[:], s_use[:, j, :], g_banks[j][:],
                                     start=j == 0, stop=j == NBANK - 1)
            for c in range(NCHAIN):
                eng(kn["add_e"][c]).tensor_add(
                    z_st[c][:, (t + 1) % NSLOT, :], e_ps[:, c, :],
                    z_st[c][:, t % NSLOT, :]
                )
            drain_out(t)

        def step_split(t, lo):
            """per-chain emission (f32r phase: latency-bound; the chains
            form two independent pipelines).  PSUM tiles keep the fused
            shape; each chain uses its half-view."""
            h_tile = hpool.tile([MLP_W, NCHAIN, Bc], f32r if lo else f32,
                                tag="h_r" if lo else "h_f", name="h_tile")
            f_ps = [
                psum_f.tile([128, NCHAIN, Bc], f32, tag=f"f_ps{j}",
                            bufs=1, name=f"f_ps{j}")
                for j in range(NBANK)
            ]
            e_ps = psum_e.tile([HID1, NCHAIN, Bc], f32, tag="e_ps",
                               name="e_ps")
            dx_blk = dx_blks[t // DXB]
            w2_use = w2_13 if lo else w2_sb
            s_use = s_sb_r if lo else s_sb
            rsplit = not lo and kn["redsplit"]
            mm2fuse = kn["mm2fuse"]
            if mm2fuse:
                # one wide mm2 per bank (both chains) after both relus;
                # everything downstream stays per-chain
                for c in range(NCHAIN):
                    h_ps = psum_h.tile([MLP_W, Bc], f32, tag="h_ps",
                                       name="h_ps")
                    nc.tensor.matmul(
                        h_ps[:], w1b_sb[:],
                        z_st[c][0 : HID + 1, t % NSLOT, :],
                        start=True, stop=True
                    )
                    relu_op(kn["relu_l"][c], h_tile[:, c, :], h_ps[:])
                for j in range(NBANK):
                    nc.tensor.matmul(f_ps[j][:], w2_use[:, j, :], h_tile[:],
                                     start=True, stop=True)
            for c in range(NCHAIN):
                cs = slice(c * Bc, (c + 1) * Bc)
                if not mm2fuse:
                    h_ps = psum_h.tile([MLP_W, Bc], f32, tag="h_ps",
                                       name="h_ps")
                    nc.tensor.matmul(
                        h_ps[:], w1b_sb[:],
                        z_st[c][0 : HID + 1, t % NSLOT, :],
                        start=True, stop=True
                    )
                    relu_op(kn["relu_l"][c], h_tile[:, c, :], h_ps[:])
                g_los = []
                for j in range(NBANK):
                    if not mm2fuse:
                        nc.tensor.matmul(f_ps[j][:, c, :], w2_use[:, j, :],
                                         h_tile[:, c, :], start=True,
                                         stop=True)
                    f_sb = fpool.tile([128, Bc], f16 if lo else f32,
                                      tag=f"f_{c}_{j}", name=f"f_sb{c}_{j}")
                    nc.scalar.activation(
                        f_sb[:], f_ps[j][:, c, :], AF.Tanh,
                        bias=b2_sb[:, j : j + 1]
                    )
                    g_sb = gpool.tile([128, Bc], f32r if lo or rsplit else f32,
                                      tag=f"g_{c}_{j}", name=f"g_sb{c}_{j}")
                    eng(kn["gmul_l"][c][j]).tensor_mul(
                        g_sb[:], f_sb[:], dx_blk[:, t % DXB, cs]
                    )
                    if rsplit:
                        # g_sb is g_hi = round11(f*dx); recompute the
                        # product exactly and subtract for the lo term
                        g2 = gpool.tile([128, Bc], f32, tag=f"g2_{c}_{j}",
                                        name=f"g2_{c}_{j}")
                        eng(kn["ghi_e"][j] if kn["ghi_e"][j] != "scalar"
                            else "vector").tensor_mul(
                            g2[:], f_sb[:], dx_blk[:, t % DXB, cs]
                        )
                        g_lo = gpool.tile([128, Bc], f32r, tag=f"glo_{c}_{j}",
                                          name=f"glo_{c}_{j}")
                        eng(kn["glo_e"][j]).tensor_sub(g_lo[:], g2[:], g_sb[:])
                        g_los.append(g_lo)
                        nc.tensor.matmul(e_ps[:, c, :], s_sb_r[:, j, :],
                                         g_sb[:], start=j == 0, stop=False)
                    else:
                        nc.tensor.matmul(e_ps[:, c, :], s_use[:, j, :],
                                         g_sb[:], start=j == 0,
                                         stop=j == NBANK - 1)
                if rsplit:
                    for j in range(NBANK):
                        nc.tensor.matmul(e_ps[:, c, :], s_sb_r[:, j, :],
                                         g_los[j][:], start=False,
                                         stop=j == NBANK - 1)
                eng(kn["add_e"][c]).tensor_add(
                    z_st[c][:, (t + 1) % NSLOT, :], e_ps[:, c, :],
                    z_st[c][:, t % NSLOT, :]
                )
            drain_out(t)

        h_box = [None]

        def new_h(k):
            lo = k > kn["t0"]
            return hpool.tile([MLP_W, NCHAIN, Bc], f32r if lo else f32,
                              tag="h_r" if lo else "h_f", name="h_tile")

        def skew_c_head(c, t, dst):
            """mm1 + relu for chain c's step t into an explicit h view"""
            h_ps = psum_h.tile([MLP_W, Bc], f32, tag="h_ps", name="h_ps")
            nc.tensor.matmul(
                h_ps[:], w1b_sb[:], z_st[c][0 : HID + 1, t % NSLOT, :],
                start=True, stop=True
            )
            relu_op(kn["relu_l"][c], dst, h_ps[:])

        def skew_tail(k, e_ps):
            """adds + drains + next step's c1 head (h for super-step k+1)"""
            if k < NSTEP:
                eng(kn["add_e"][0]).tensor_add(
                    z_st[0][:, (k + 1) % NSLOT, :], e_ps[:, 0, :],
                    z_st[0][:, k % NSLOT, :]
                )
                drain_chain(0, k + 1)
            if k >= 1:
                eng(kn["add_e"][1]).tensor_add(
                    z_st[1][:, k % NSLOT, :], e_ps[:, 1, :],
                    z_st[1][:, (k - 1) % NSLOT, :]
                )
                drain_chain(1, k)
            h_next = new_h(k + 1)
            if k < NSTEP:
                skew_c_head(1, k, h_next[:, 1, :])
            h_box[0] = h_next

        def step_skew(k):
            """wide super-step: c0@k and c1@(k-1) share every wide op (the
            host pre-skews dx so each chain half reads its own step)"""
            lo = k > kn["t0"]
            h_cur = h_box[0]
            if k < NSTEP:
                skew_c_head(0, k, h_cur[:, 0, :])
            dx_sb = dx_blks[k // DXB][:, k % DXB, :]
            w2_use = w2_13 if lo else w2_sb
            for j in range(NBANK):
                f_ps = psum_f.tile([128, NCHAIN, Bc], f32, tag=f"f_ps{j}",
                                   bufs=1, name=f"f_ps{j}")
                nc.tensor.matmul(f_ps[:], w2_use[:, j, :], h_cur[:],
                                 start=True, stop=True)
                f_sb = fpool.tile([128, NCHAIN, Bc], f16 if lo else f32,
                                  tag=f"f_r{j}" if lo else f"f_f{j}",
                                  name=f"f_sb{j}")
                nc.scalar.activation(
                    f_sb[:], f_ps[:], AF.Tanh, bias=b2_sb[:, j : j + 1]
                )
                g_sb = gpool.tile([128, NCHAIN, Bc], f32r if lo else f32,
                                  tag=f"g_r{j}" if lo else f"g_f{j}",
                                  name=f"g_sb{j}")
                eng(kn["gmul_e"][j]).tensor_mul(g_sb[:], f_sb[:], dx_sb[:])
                g_banks[j] = g_sb
            e_ps = psum_e.tile([HID1, NCHAIN, Bc], f32, tag="e_ps",
                               name="e_ps")
            s_use = s_sb_r if lo else s_sb
            for j in range(NBANK):
                nc.tensor.matmul(e_ps[:], s_use[:, j, :], g_banks[j][:],
                                 start=j == 0, stop=j == NBANK - 1)
            skew_tail(k, e_ps)

        def step_skew_boundary(k):
            """phase-boundary super-step (k == t0): per-chain emission so
            c0@k runs f32r while c1@(k-1) runs fp32 — keeps both chains
            bit-identical to the non-skew hybrid schedule"""
            h_cur = h_box[0]  # f32 tile: c1's half is valid fp32 h
            h_bnd = hpool.tile([MLP_W, Bc], f32r, tag="h_bnd", name="h_bnd")
            skew_c_head(0, k, h_bnd[:])
            dx_blk = dx_blks[k // DXB]
            e_ps = psum_e.tile([HID1, NCHAIN, Bc], f32, tag="e_ps",
                               name="e_ps")
            f_ps = [
                psum_f.tile([128, NCHAIN, Bc], f32, tag=f"f_ps{j}",
                            bufs=1, name=f"f_ps{j}")
                for j in range(NBANK)
            ]
            for c, clo in ((0, True), (1, False)):
                cs = slice(c * Bc, (c + 1) * Bc)
                w2_use = w2_13 if clo else w2_sb
                s_use = s_sb_r if clo else s_sb
                h_in = h_bnd[:] if c == 0 else h_cur[:, 1, :]
                for j in range(NBANK):
                    nc.tensor.matmul(f_ps[j][:, c, :], w2_use[:, j, :],
                                     h_in, start=True, stop=True)
                    f_sb = fpool.tile([128, Bc], f16 if clo else f32,
                                      tag=f"f_{c}_{j}", name=f"f_sb{c}_{j}")
                    nc.scalar.activation(
                        f_sb[:], f_ps[j][:, c, :], AF.Tanh,
                        bias=b2_sb[:, j : j + 1]
                    )
                    g_sb = gpool.tile([128, Bc], f32r if clo else f32,
                                      tag=f"g_{c}_{j}", name=f"g_sb{c}_{j}")
                    eng(kn["gmul_l"][c][j]).tensor_mul(
                        g_sb[:], f_sb[:], dx_blk[:, k % DXB, cs]
                    )
                    nc.tensor.matmul(e_ps[:, c, :], s_use[:, j, :], g_sb[:],
                                     start=j == 0, stop=j == NBANK - 1)
            skew_tail(k, e_ps)

        def dma_dx(blk):
            if blk >= NDXBLK:
                return
            dx_sb = dxpool.tile([128, DXB, B], f32, tag="dx", name="dx_sb")
            nc.sync.dma_start(
                dx_sb[:],
                dxt[blk][None].to_broadcast([128 // IN_CH, IN_CH, DXB, B]),
            )
            dx_blks[blk] = dx_sb
            if blk - DXBLK_PREFETCH - 1 in dx_blks:
                del dx_blks[blk - DXBLK_PREFETCH - 1]

        def scan_body():
            init_chains()
            dx_blks.clear()
            for blk in range(DXBLK_PREFETCH):
                dma_dx(blk)
            if kn["skew"]:
                h0 = new_h(0)
                nc.vector.memset(h0[:, 1, :], 0.0)
                h_box[0] = h0
                for k in range(NSTEP + 1):
                    if k % DXB == 0:
                        dma_dx(k // DXB + DXBLK_PREFETCH)
                    if k == kn["t0"]:
                        step_skew_boundary(k)
                    else:
                        step_skew(k)
                return
            for t in range(NSTEP):
                lo = t >= kn["t0"]
                if t % DXB == 0:
                    dma_dx(t // DXB + DXBLK_PREFETCH)
                if kn["lsplit"] if lo else kn["esplit"]:
                    step_split(t, lo)
                else:
                    step_fused(t, lo)

        if repeat == 1:
            scan_body()
        else:
            # hardware loop: trip count is a runtime scalar, so timing
            # amplification costs no extra instructions
            with tc.For_i(0, repeat):
                scan_body()

    nc.compile()
    return nc


def _prep_host(u0, coeffs, W1, b1, W2, b2, Wi, bi, Wr, br, skew=True):
    f32 = np.float32

    u0t_full = np.empty((IN_CH, B_FULL), f32)
    u0t_full[: IN_CH - 1] = u0.T
    u0t_full[IN_CH - 1] = 1.0

    dX = (coeffs[:, 1:] - coeffs[:, :-1]).astype(f32)  # [B_FULL, NSTEP, IN_CH]
    dxt_step = dX.transpose(1, 2, 0)  # [NSTEP, 8, B_FULL]
    dxt_pad = np.zeros((NDXBLK * DXB, IN_CH, B_FULL), f32)
    if skew:
        # pre-skewed: super-step k serves chain0@k and chain1@(k-1), so
        # each core-shard's chain-1 half (columns 256:512 of every 512)
        # carries the PREVIOUS step's dx.  Chain halves repeat per core.
        is_c1 = (np.arange(B_FULL) % (2 * Bc)) >= Bc
        for k in range(NSTEP + 1):
            if k < NSTEP:
                dxt_pad[k][:, ~is_c1] = dxt_step[k][:, ~is_c1]
            if k >= 1:
                dxt_pad[k][:, is_c1] = dxt_step[k - 1][:, is_c1]
    else:
        dxt_pad[:NSTEP] = dxt_step
    # [NDXBLK, 8, DXB, B_FULL]: one DMA block covers DXB scan steps
    dxt_small = np.ascontiguousarray(
        dxt_pad.reshape(NDXBLK, DXB, IN_CH, B_FULL).transpose(0, 2, 1, 3)
    )

    # z columns: 0..63 state, 64 const-one, 65 readout
    wi_mat = np.zeros((IN_CH, HID1), f32)
    wi_mat[: IN_CH - 1, :HID] = Wi
    wi_mat[IN_CH - 1, :HID] = bi
    wi_mat[IN_CH - 1, ROW_ONE] = 1.0
    wi_mat[: IN_CH - 1, ROW_OUT] = (Wi @ Wr)[:, 0]
    wi_mat[IN_CH - 1, ROW_OUT] = float(bi @ Wr[:, 0] + br[0])

    # mm1 stationary: [W1; b1] against moving z[0:65] (row 64 == 1.0)
    w1b = np.empty((HID + 1, MLP_W), f32)
    w1b[:HID] = W1
    w1b[HID] = b1

    w2_banks = np.ascontiguousarray(W2.reshape(MLP_W, NBANK, 128))
    b2_banks = np.ascontiguousarray(b2.reshape(NBANK, 128).T)

    p = np.arange(128)
    s_full = np.zeros((128, NBANK, HID1), f32)
    for j in range(NBANK):
        s_full[p, j, 16 * j + p // IN_CH] = 1.0
        s_full[p, j, ROW_OUT] = Wr[16 * j + p // IN_CH, 0]

    return {
        "u0t": u0t_full,
        "dxt": dxt_small,
        "w1b": w1b,
        "w2": w2_banks.astype(f32),
        "b2": b2_banks.astype(f32),
        "wi": wi_mat,
        "smat": s_full,
    }


def _make_in_maps(full):
    in_maps = []
    for c in range(N_CORES):
        sl = slice(c * B, (c + 1) * B)
        in_maps.append(
            {
                "u0t": np.ascontiguousarray(full["u0t"][:, sl]),
                "dxt": np.ascontiguousarray(full["dxt"][:, :, :, sl]),
                "w1b": full["w1b"],
                "w2": full["w2"],
                "b2": full["b2"],
                "wi": full["wi"],
                "smat": full["smat"],
            }
        )
    return in_maps


def kernel(u0, coeffs, W1, b1, W2, b2, Wi, bi, Wr, br, repeat=None, knobs=None):
    from concourse.bass_utils import run_bass_kernel_spmd

    skew = (knobs or {}).get("skew", KNOBS["skew"])
    full = _prep_host(
        np.asarray(u0, np.float32), np.asarray(coeffs, np.float32),
        np.asarray(W1, np.float32), np.asarray(b1, np.float32),
        np.asarray(W2, np.float32), np.asarray(b2, np.float32),
        np.asarray(Wi, np.float32), np.asarray(bi, np.float32),
        np.asarray(Wr, np.float32).reshape(HID, OUT),
        np.asarray(br, np.float32).reshape(OUT),
        skew=skew,
    )
    in_maps = _make_in_maps(full)

    rep = REPEAT if repeat is None else repeat
    key = ("nc", rep, repr(sorted(knobs.items())) if knobs else None)
    if key not in _CACHE:
        _CACHE[key] = _build_bass(rep, knobs)
    nc = _CACHE[key]

    res = run_bass_kernel_spmd(nc, in_maps, core_ids=list(range(N_CORES)))
    outs = res.results

    out_full = np.empty((B_FULL, T, OUT), np.float32)
    for c in range(N_CORES):
        out_full[c * B : (c + 1) * B, :, 0] = outs[c]["outp"].T
    return out_full
